# revision 47
# baseline (speedup 1.0000x reference)
"""Trainium2 Bass kernel for CrossAttention (silu-scored, masked) sharded over
8 NeuronCores.

Problem (full shapes):
    query/key/value: [2, 2048, 1024] f32, mask: [2, 1, 2048, 2048] int32
    out = silu(mask((q @ k.T) * scale)) @ v heads-merged @ Wo.T

Sharding: core c handles batch b = c // 4 and heads 4*(c%4) .. 4*(c%4)+3
(data parallel on B, tensor parallel on heads).  Each core computes a
row-parallel partial of the O-projection; the host sums the 4 partials per
batch.  No cross-device communication.

Per-core program (all matmul operands bf16, f32 PSUM accumulate).  The 4
local heads form 2 pairs (even head on partitions 0-63, odd on 64-127 of
the shared qt/kt tiles).  Score matmuls are two K=64 row-group matmuls
(tile_position (0,0)/(64,0)) and context matmuls two M=64 col-group
matmuls ((0,0)/(0,64)), emitted pair-adjacent so they run CONCURRENTLY on
disjoint halves of the PE array — half the PE wall-clock of the padded
full-array formulation.

ACT (silu) is the roofline engine: 128 ACTIVATEs x ~1.1us ≈ 145us busy
(FD=1024 is forced by PSUM: score tiles must double-buffer next to the
ctx accumulator and a projection/O-proj scratch slot in 8 banks).  All
four (pair, sq-half) attention passes are fused into ONE 64-step
software pipeline so nothing gates ACT:
  - mask + scores for step i+1 are emitted before ctx of step i, across
    pass boundaries too (no boundary stall)
  - ctx runs 2 steps deferred (both attn tiles ready -> the two M=64
    col-group matmuls run concurrently)
  - projection work that isn't needed up front (v, k/q second halves,
    q col-half 2, O-proj head) is chopped into ~1-2us "filler" units
    paced into the pipeline after their producers / before consumers
  - masks/outputs ride the sync DMA ring; the scalar ring is quiet after
    the early x loads so the ACT sequencer only runs ACTIVATEs
  - dep-free warm-up matmuls bridge the DMA head so the projections hit
    the PE at 2.4 GHz (HAM un-throttled) instead of 1.2.
"""

import numpy as np
import ml_dtypes

B = 2
S = 2048
HID = 1024
HEADS = 16
DH = 64
N_CORES = 8
GROUPS = 4
NH_LOC = HEADS // GROUPS   # 4 heads per core
DLOC = NH_LOC * DH         # 256 local features
SCALE = DH ** -0.5

F32 = np.float32
BF16 = ml_dtypes.bfloat16

_COMPILED = {}


def build_program():
    import concourse.bass as bass
    import concourse.tile as tile
    from concourse import bacc, mybir
    from concourse.masks import make_identity

    f32 = mybir.dt.float32
    bf16 = mybir.dt.bfloat16

    nc = bacc.Bacc("TRN2", target_bir_lowering=False, debug=False,
                   enable_asserts=False, num_devices=N_CORES)

    xq = nc.dram_tensor("xq", [HID, S], bf16, kind="ExternalInput").ap()
    xk = nc.dram_tensor("xk", [HID, S], bf16, kind="ExternalInput").ap()
    xv = nc.dram_tensor("xv", [HID, S], bf16, kind="ExternalInput").ap()
    mk = nc.dram_tensor("mk", [S, S], bf16, kind="ExternalInput").ap()
    wq = nc.dram_tensor("wq", [HID, DLOC], bf16, kind="ExternalInput").ap()
    wk = nc.dram_tensor("wk", [HID, DLOC], bf16, kind="ExternalInput").ap()
    wv = nc.dram_tensor("wv", [HID, DLOC], bf16, kind="ExternalInput").ap()
    wo = nc.dram_tensor("wo", [DLOC, HID], bf16, kind="ExternalInput").ap()
    bq = nc.dram_tensor("bq", [DLOC, 1], f32, kind="ExternalInput").ap()
    bk = nc.dram_tensor("bk", [DLOC, 1], f32, kind="ExternalInput").ap()
    bv = nc.dram_tensor("bv", [DLOC, 1], f32, kind="ExternalInput").ap()
    out1 = nc.dram_tensor("out1", [S, HID], bf16, kind="ExternalOutput").ap()

    SILU = mybir.ActivationFunctionType.Silu
    MUL = mybir.AluOpType.mult
    ADD = mybir.AluOpType.add

    with tile.TileContext(nc) as tc:
        with (
            tc.tile_pool(name="res", bufs=1) as res,
            tc.tile_pool(name="io", bufs=8) as io,
            tc.tile_pool(name="wp", bufs=3) as wp,
            # PSUM: sc 2x[128,1024] (4 banks) + acc 1x[128,1024] (2) +
            # pp 2x[128,512] (2) = 8 banks exactly.
            tc.tile_pool(name="ps", bufs=2, space="PSUM") as ps,
            tc.tile_pool(name="attp", bufs=4) as attp,
            tc.tile_pool(name="mpool", bufs=4) as mpool,
            tc.tile_pool(name="vt", bufs=1) as vtp,
            tc.tile_pool(name="oev", bufs=4) as oev,
        ):
            # ---- resident SBUF tensors ----
            qt = [res.tile([128, S], bf16, tag=f"qt{t}", name=f"qt{t}") for t in range(2)]
            kt = [res.tile([128, S], bf16, tag=f"kt{t}", name=f"kt{t}") for t in range(2)]
            # vp quarter qi covers sk-tiles 4qi..4qi+3; col layout within:
            # (j%4)*256 + t*128 + hp*64 + d
            vp = [res.tile([128, 1024], bf16, tag=f"vp{qi}", name=f"vp{qi}") for qi in range(4)]
            # ctxt[t][sqh]: rows = pair-t features, cols = sq half
            ctxt = [[res.tile([128, 1024], bf16, tag=f"cx{t}{sqh}", name=f"cx{t}{sqh}")
                     for sqh in range(2)] for t in range(2)]
            wo_sb = [res.tile([128, HID], bf16, tag=f"wo{t}", name=f"wo_sb{t}") for t in range(2)]
            ident = res.tile([128, 128], bf16, tag="ident", name="ident")
            b_sb = {}
            for nm, srcb in (("bq", bq), ("bk", bk), ("bv", bv)):
                b_sb[nm] = [res.tile([128, 1], f32, tag=f"{nm}{m}", name=f"{nm}_sb{m}") for m in range(2)]
                for m in range(2):
                    nc.scalar.dma_start(out=b_sb[nm][m][:, :], in_=srcb[m * 128:(m + 1) * 128, :])
            for t in range(2):
                nc.scalar.dma_start(out=wo_sb[t][:, :], in_=wo[t * 128:(t + 1) * 128, :])
            make_identity(nc, ident[:, :])

            # ---- x staging tiles ----
            # One big DMA per tensor: a single InstDMACopy fans out across
            # all 16 SDMA engines, so the whole tensor lands faster than 8
            # serialized ring entries.  Callers keep per-chunk addressing
            # via AP views of the single tile.
            def load_x(x_ap, nm, dmae, cols=None):
                w = S if cols is None else 1024
                xt = io.tile([128, 8 * w], bf16, tag=f"{nm}", name=nm, bufs=1)
                src = x_ap.rearrange("(k p) s -> p k s", p=128)
                if cols is not None:
                    src = src[:, :, cols * 1024:(cols + 1) * 1024]
                dmae.dma_start(
                    out=xt[:, :].rearrange("p (k s) -> p k s", k=8), in_=src
                )
                return [xt[:, k * w:(k + 1) * w] for k in range(8)]

            def load_w(w_ap, nm):
                w_sb = wp.tile([128, 8 * DLOC], bf16, tag="w", name=f"w_{nm}")
                nc.scalar.dma_start(
                    out=w_sb[:, :].rearrange("p (k m) -> p k m", k=8),
                    in_=w_ap.rearrange("(k p) m -> p k m", p=128),
                )
                return w_sb

            wk_sb = load_w(wk, "k")
            wq_sb = load_w(wq, "q")
            wv_sb = load_w(wv, "v")
            xk_t = load_x(xk, "xk", nc.sync)
            xqA = load_x(xq, "xqA", nc.scalar, cols=0)
            xv_t = load_x(xv, "xv", nc.sync)
            xqB = load_x(xq, "xqB", nc.scalar, cols=1)

            # ---- projection pass emitters ----------------------------------
            # proj n-pair: x chunks stream k-inner so matmuls start as DMA
            # lands; returns filler-unit closures of ~4-8 matmuls each.
            def proj_units(w_sb, x_tiles, dst, m, npair, bias, xoff=0, nm=""):
                st = {}

                def mms(klo, khi):
                    def f():
                        if "pacc" not in st:
                            st["pacc"] = [
                                ps.tile([128, 512], f32, tag="pp", name=f"pj{nm}{m}{n}")
                                for n in npair
                            ]
                        for k in range(klo, khi):
                            for i, n in enumerate(npair):
                                nc.tensor.matmul(
                                    st["pacc"][i][:, :],
                                    lhsT=w_sb[:, k * DLOC + m * 128: k * DLOC + (m + 1) * 128],
                                    rhs=x_tiles[k][:, (n - xoff) * 512:(n - xoff + 1) * 512],
                                    start=(k == 0), stop=(k == 7),
                                )
                    return f

                def evac():
                    for i, n in enumerate(npair):
                        nc.vector.tensor_scalar(
                            out=dst[m][:, n * 512:(n + 1) * 512],
                            in0=st["pacc"][i][:, :],
                            scalar1=1.0,
                            scalar2=bias[m][:, 0:1],
                            op0=MUL, op1=ADD,
                        )
                return [mms(0, 4), mms(4, 8), evac]

            # v projection, orientation A (features on partitions) + PE
            # transpose into the pair-packed [sk, feat] layout.  Returns two
            # unit-lists (one per n-pair), each with its transposes inline
            # right after the evac so vp quarters become valid (in trace
            # order) as early as possible.
            vt_bf = [None, None]

            def v_units(m):
                vt_bf[m] = vtp.tile([128, S], bf16, tag="vt", name=f"vt{m}")

                def transp(jc):
                    def f():
                        tr = ps.tile([128, 128], bf16, tag="pp", name=f"tr{m}{jc}")
                        nc.tensor.transpose(
                            tr[:, :], vt_bf[m][:, jc * 128:(jc + 1) * 128], ident[:, :]
                        )
                        nc.vector.tensor_copy(
                            out=vp[jc // 4][:, (jc % 4) * 256 + m * 128:
                                            (jc % 4) * 256 + m * 128 + 128],
                            in_=tr[:, :],
                        )
                    return f

                halves = []
                for hi, npair in enumerate(((0, 1), (2, 3))):
                    us = proj_units(wv_sb, xv_t, vt_bf, m, npair, b_sb["bv"], nm="v")
                    us += [transp(jc) for jc in range(hi * 8, hi * 8 + 8)]
                    halves.append(us)
                return halves

            def o_units(mbs):
                units = []

                def one(mb, n2):
                    def f():
                        sqh, col = mb // 8, (mb % 8) * 128
                        po = ps.tile([128, 512], f32, tag="pp", name=f"po{mb}{n2}")
                        for t in range(2):
                            nc.tensor.matmul(
                                po[:, :],
                                lhsT=ctxt[t][sqh][:, col:col + 128],
                                rhs=wo_sb[t][:, n2 * 512:(n2 + 1) * 512],
                                start=(t == 0), stop=(t == 1),
                            )
                        ev = oev.tile([128, 512], bf16, tag="oev", name=f"ev{mb}{n2}")
                        if mb < 8:
                            nc.vector.tensor_copy(out=ev[:, :], in_=po[:, :])
                            dmae = nc.sync
                        else:
                            nc.scalar.copy(out=ev[:, :], in_=po[:, :])
                            dmae = nc.sync if n2 == 0 else nc.scalar
                        dmae.dma_start(
                            out=out1[mb * 128:(mb + 1) * 128, n2 * 512:(n2 + 1) * 512],
                            in_=ev[:, :],
                        )
                    return f
                for mb in mbs:
                    for n2 in range(2):
                        units.append(one(mb, n2))
                return units

            # ---- attention pass (software-pipelined) -----------------------
            def emit_scores(t, sqh, j):
                sA = ps.tile([128, 1024], f32, tag="sc", name="sA")
                sB = ps.tile([128, 1024], f32, tag="sc", name="sB")
                # pair-adjacent: the two K=64 matmuls run concurrently on
                # row-groups 0-1 / 2-3 when both PSUM slots are free.
                for c in range(2):
                    for s_, lo in ((sA, 0), (sB, 64)):
                        nc.tensor.matmul(
                            s_[:, c * 512:(c + 1) * 512],
                            lhsT=kt[t][lo:lo + 64, j * 128:(j + 1) * 128],
                            rhs=qt[t][lo:lo + 64, sqh * 1024 + c * 512: sqh * 1024 + (c + 1) * 512],
                            start=True, stop=True,
                        )
                return sA, sB

            def emit_mask(sqh, j):
                mt = mpool.tile([128, 1024], bf16, tag="mt", name="mt")
                nc.sync.dma_start(
                    out=mt[:, :],
                    in_=mk[j * 128:(j + 1) * 128, sqh * 1024:(sqh + 1) * 1024],
                )
                return mt

            # One fused 64-step pipeline over (t, sqh, j): mask + scores are
            # emitted one step ahead ACROSS pass boundaries, so ACT never
            # waits at a boundary; ctx runs 2 steps deferred (both attn
            # tiles ready -> the M=64 pair runs concurrently); per-pass
            # filler units pace ~evenly through their 16 steps.
            def attn_all(seg_fillers):
                steps = [(t, sqh, j) for t in range(2) for sqh in range(2)
                         for j in range(16)]
                accs = {}
                ctx_q = []

                def emit_ctx(t, sqh, j, atnA, atnB):
                    acc = accs[(t, sqh)]
                    vbase = (j % 4) * 256 + t * 128
                    for c in range(2):
                        for hp, atn in ((0, atnA), (1, atnB)):
                            nc.tensor.matmul(
                                acc[hp * 64:(hp + 1) * 64, c * 512:(c + 1) * 512],
                                lhsT=vp[j // 4][:, vbase + hp * 64: vbase + hp * 64 + 64],
                                rhs=atn[:, c * 512:(c + 1) * 512],
                                start=(j == 0), stop=(j == 15),
                            )
                    if j == 15:
                        nc.vector.tensor_copy(out=ctxt[t][sqh][:, :], in_=acc[:, :])

                t0, sqh0, _ = steps[0]
                accs[(t0, sqh0)] = ps.tile([128, 1024], f32, tag="acc",
                                           name=f"acc{t0}{sqh0}", bufs=1)
                mt = emit_mask(sqh0, 0)
                sAB = emit_scores(t0, sqh0, 0)
                for i, (t, sqh, j) in enumerate(steps):
                    fillers = seg_fillers[2 * t + sqh]
                    sA, sB = sAB
                    attA = attp.tile([128, 1024], bf16, tag="att", name="attA", bufs=5)
                    attB = attp.tile([128, 1024], bf16, tag="att", name="attB", bufs=5)
                    nc.scalar.activation(attA[:, :], sA[:, :], SILU)
                    nc.scalar.activation(attB[:, :], sB[:, :], SILU)
                    atnA = attp.tile([128, 1024], bf16, tag="attn", name="atnA", bufs=11)
                    atnB = attp.tile([128, 1024], bf16, tag="attn", name="atnB", bufs=11)
                    nc.vector.tensor_mul(out=atnA[:, :], in0=attA[:, :], in1=mt[:, :])
                    nc.vector.tensor_mul(out=atnB[:, :], in0=attB[:, :], in1=mt[:, :])
                    if i + 1 < len(steps):
                        nt, nsqh, nj = steps[i + 1]
                        if nj == 0:
                            accs[(nt, nsqh)] = ps.tile([128, 1024], f32, tag="acc",
                                                       name=f"acc{nt}{nsqh}", bufs=1)
                        mt = emit_mask(nsqh, nj)
                        sAB = emit_scores(nt, nsqh, nj)
                    # filler units: pace the segment's list over its steps
                    rem_iters = 16 - j
                    n_emit = (len(fillers) + rem_iters - 1) // rem_iters if fillers else 0
                    for _ in range(min(n_emit, len(fillers))):
                        fillers.pop(0)()
                    ctx_q.append((t, sqh, j, atnA, atnB))
                    if len(ctx_q) > 2:
                        emit_ctx(*ctx_q.pop(0))
                while ctx_q:
                    emit_ctx(*ctx_q.pop(0))

            # ---- emission schedule ----------------------------------------
            # HAM warm-up: dep-free matmuls on garbage SBUF fill the
            # preamble + xk-DMA window with sustained PE activity so the
            # projections run at 2.4 GHz, not 1.2.  Output is discarded
            # (scores later overwrite the slot with start=True).
            wrm = ps.tile([128, 1024], f32, tag="sc", name="warm")
            for _ in range(24):
                nc.tensor.matmul(wrm[:, 0:512], lhsT=ident[:, :],
                                 rhs=kt[0][0:128, 0:512], start=True, stop=True)

            # head: k m0 + q m0 (cols 0-1023) gate the first scores
            for u in proj_units(wk_sb, xk_t, kt, 0, (0, 1), b_sb["bk"], nm="k"):
                u()
            for u in proj_units(wk_sb, xk_t, kt, 0, (2, 3), b_sb["bk"], nm="k"):
                u()
            for u in proj_units(wq_sb, xqA, qt, 0, (0, 1), b_sb["bq"], nm="q"):
                u()

            v0a, v0b = v_units(0)
            v1a, v1b = v_units(1)
            # head-start v m0's first mms before the attention stream
            v0a[0]()
            v0a[1]()
            # filler load balanced across segments.  Ordering rule: units
            # whose output is read by a NEXT-segment boundary score (k/q
            # evacs) come early; transposes only consumed by late deferred
            # ctx go last.  (v-pair-1's second half is first consumed by
            # ctx(1,0,8) around step 42, so it can pace through segment 2.)
            q023 = proj_units(wq_sb, xqB, qt, 0, (2, 3), b_sb["bq"], xoff=2, nm="q")
            q123 = proj_units(wq_sb, xqB, qt, 1, (2, 3), b_sb["bq"], xoff=2, nm="q")
            f00 = v0a[2:3] + v0a[3:7] + q023 + v0a[7:] + v0b
            f01 = v1a[:7] \
                + proj_units(wk_sb, xk_t, kt, 1, (0, 1), b_sb["bk"], nm="k") \
                + proj_units(wq_sb, xqA, qt, 1, (0, 1), b_sb["bq"], nm="q") \
                + v1a[7:]
            f10 = proj_units(wk_sb, xk_t, kt, 1, (2, 3), b_sb["bk"], nm="k") \
                + v1b[:7] + q123 + v1b[7:]
            f11 = [lambda: None] * 4 + o_units(range(0, 8))
            attn_all([f00, f01, f10, f11])
            for u in o_units(range(8, 16)):
                u()

    nc.compile()
    return nc


def get_program():
    if "nc" not in _COMPILED:
        _COMPILED["nc"] = build_program()
    return _COMPILED["nc"]


def make_in_maps(query, key, value, mask, Wq, bq, Wk, bk, Wv, bv, Wo):
    """Host-side sharding/layout prep: one input map per core."""
    query = np.asarray(query, dtype=F32)
    key = np.asarray(key, dtype=F32)
    value = np.asarray(value, dtype=F32)
    mask = np.asarray(mask)
    in_maps = []
    maskT = [np.ascontiguousarray(mask[b, 0].T).astype(BF16) for b in range(B)]
    xqT = [np.ascontiguousarray(query[b].T).astype(BF16) for b in range(B)]
    xkT = [np.ascontiguousarray(key[b].T).astype(BF16) for b in range(B)]
    xvT = [np.ascontiguousarray(value[b].T).astype(BF16) for b in range(B)]
    for c in range(N_CORES):
        b = c // GROUPS
        g = c % GROUPS
        rs = slice(g * DLOC, (g + 1) * DLOC)
        in_maps.append({
            "xq": xqT[b],
            "xk": xkT[b],
            "xv": xvT[b],
            "mk": maskT[b],
            "wq": np.ascontiguousarray((SCALE * np.asarray(Wq, F32))[rs, :].T).astype(BF16),
            "wk": np.ascontiguousarray(np.asarray(Wk, F32)[rs, :].T).astype(BF16),
            "wv": np.ascontiguousarray(np.asarray(Wv, F32)[rs, :].T).astype(BF16),
            "wo": np.ascontiguousarray(np.asarray(Wo, F32)[:, rs].T).astype(BF16),
            "bq": (SCALE * np.asarray(bq, F32)[rs]).reshape(DLOC, 1),
            "bk": np.asarray(bk, F32)[rs].reshape(DLOC, 1),
            "bv": np.asarray(bv, F32)[rs].reshape(DLOC, 1),
        })
    return in_maps


def run_on_device(in_maps, trace=False, tmpdir=None):
    from concourse.bass_utils import run_bass_kernel_spmd
    nc = get_program()
    kwargs = {}
    if trace:
        kwargs.update(trace=True, tmpdir=tmpdir)
    return run_bass_kernel_spmd(nc, in_maps, list(range(N_CORES)), **kwargs)


def assemble_output(results):
    out = np.zeros((B, S, HID), dtype=F32)
    for c in range(N_CORES):
        out[c // GROUPS] += results[c]["out1"].astype(F32)
    return out


def kernel(query, key, value, mask, Wq, bq, Wk, bk, Wv, bv, Wo):
    in_maps = make_in_maps(query, key, value, mask, Wq, bq, Wk, bk, Wv, bv, Wo)
    res = run_on_device(in_maps)
    return assemble_output(res.results)


# revision 48
# speedup vs baseline: 1.0306x; 1.0306x over previous
"""Trainium2 Bass kernel for CrossAttention (silu-scored, masked) sharded over
8 NeuronCores.

Problem (full shapes):
    query/key/value: [2, 2048, 1024] f32, mask: [2, 1, 2048, 2048] int32
    out = silu(mask((q @ k.T) * scale)) @ v heads-merged @ Wo.T

Sharding: core c handles batch b = c // 4 and heads 4*(c%4) .. 4*(c%4)+3
(data parallel on B, tensor parallel on heads).  Each core computes a
row-parallel partial of the O-projection; the host sums the 4 partials per
batch.  No cross-device communication.

Per-core program (all matmul operands bf16, f32 PSUM accumulate).  The 4
local heads form 2 pairs (even head on partitions 0-63, odd on 64-127 of
the shared qt/kt tiles).  Score matmuls are two K=64 row-group matmuls
(tile_position (0,0)/(64,0)) and context matmuls two M=64 col-group
matmuls ((0,0)/(0,64)), emitted pair-adjacent so they run CONCURRENTLY on
disjoint halves of the PE array — half the PE wall-clock of the padded
full-array formulation.

ACT (silu) is the roofline engine: 128 ACTIVATEs x ~1.1us ≈ 145us busy
(FD=1024 is forced by PSUM: score tiles must double-buffer next to the
ctx accumulator and a projection/O-proj scratch slot in 8 banks).  All
four (pair, sq-half) attention passes are fused into ONE 64-step
software pipeline so nothing gates ACT:
  - mask + scores for step i+1 are emitted before ctx of step i, across
    pass boundaries too (no boundary stall)
  - ctx runs 2 steps deferred (both attn tiles ready -> the two M=64
    col-group matmuls run concurrently)
  - projection work that isn't needed up front (v, k/q second halves,
    q col-half 2, O-proj head) is chopped into ~1-2us "filler" units
    paced into the pipeline after their producers / before consumers
  - masks/outputs ride the sync DMA ring; the scalar ring is quiet after
    the early x loads so the ACT sequencer only runs ACTIVATEs
  - dep-free warm-up matmuls bridge the DMA head so the projections hit
    the PE at 2.4 GHz (HAM un-throttled) instead of 1.2.
"""

import numpy as np
import ml_dtypes

B = 2
S = 2048
HID = 1024
HEADS = 16
DH = 64
N_CORES = 8
GROUPS = 4
NH_LOC = HEADS // GROUPS   # 4 heads per core
DLOC = NH_LOC * DH         # 256 local features
SCALE = DH ** -0.5

F32 = np.float32
BF16 = ml_dtypes.bfloat16

_COMPILED = {}


def build_program():
    import concourse.bass as bass
    import concourse.tile as tile
    from concourse import bacc, mybir
    from concourse.masks import make_identity

    f32 = mybir.dt.float32
    bf16 = mybir.dt.bfloat16

    nc = bacc.Bacc("TRN2", target_bir_lowering=False, debug=False,
                   enable_asserts=False, num_devices=N_CORES)

    xq = nc.dram_tensor("xq", [HID, S], bf16, kind="ExternalInput").ap()
    xk = nc.dram_tensor("xk", [HID, S], bf16, kind="ExternalInput").ap()
    xv = nc.dram_tensor("xv", [HID, S], bf16, kind="ExternalInput").ap()
    mk = nc.dram_tensor("mk", [S, S], bf16, kind="ExternalInput").ap()
    wq = nc.dram_tensor("wq", [HID, DLOC], bf16, kind="ExternalInput").ap()
    wk = nc.dram_tensor("wk", [HID, DLOC], bf16, kind="ExternalInput").ap()
    wv = nc.dram_tensor("wv", [HID, DLOC], bf16, kind="ExternalInput").ap()
    wo = nc.dram_tensor("wo", [DLOC, HID], bf16, kind="ExternalInput").ap()
    bq = nc.dram_tensor("bq", [DLOC, 1], f32, kind="ExternalInput").ap()
    bk = nc.dram_tensor("bk", [DLOC, 1], f32, kind="ExternalInput").ap()
    bv = nc.dram_tensor("bv", [DLOC, 1], f32, kind="ExternalInput").ap()
    out1 = nc.dram_tensor("out1", [S, HID], bf16, kind="ExternalOutput").ap()

    SILU = mybir.ActivationFunctionType.Silu
    MUL = mybir.AluOpType.mult
    ADD = mybir.AluOpType.add

    with tile.TileContext(nc) as tc:
        with (
            tc.tile_pool(name="res", bufs=1) as res,
            tc.tile_pool(name="io", bufs=8) as io,
            tc.tile_pool(name="wp", bufs=3) as wp,
            # PSUM: sc 2x[128,1024] (4 banks) + acc 1x[128,1024] (2) +
            # pp 2x[128,512] (2) = 8 banks exactly.
            tc.tile_pool(name="ps", bufs=2, space="PSUM") as ps,
            tc.tile_pool(name="attp", bufs=4) as attp,
            tc.tile_pool(name="mpool", bufs=4) as mpool,
            tc.tile_pool(name="vt", bufs=1) as vtp,
            tc.tile_pool(name="oev", bufs=4) as oev,
        ):
            # ---- resident SBUF tensors ----
            qt = [res.tile([128, S], bf16, tag=f"qt{t}", name=f"qt{t}") for t in range(2)]
            kt = [res.tile([128, S], bf16, tag=f"kt{t}", name=f"kt{t}") for t in range(2)]
            # vp quarter qi covers sk-tiles 4qi..4qi+3; col layout within:
            # (j%4)*256 + t*128 + hp*64 + d
            vp = [res.tile([128, 1024], bf16, tag=f"vp{qi}", name=f"vp{qi}") for qi in range(4)]
            # ctxt[t][sqh]: rows = pair-t features, cols = sq half
            ctxt = [[res.tile([128, 1024], bf16, tag=f"cx{t}{sqh}", name=f"cx{t}{sqh}")
                     for sqh in range(2)] for t in range(2)]
            wo_sb = [res.tile([128, HID], bf16, tag=f"wo{t}", name=f"wo_sb{t}") for t in range(2)]
            ident = res.tile([128, 128], bf16, tag="ident", name="ident")
            b_sb = {}
            for nm, srcb in (("bq", bq), ("bk", bk), ("bv", bv)):
                b_sb[nm] = [res.tile([128, 1], f32, tag=f"{nm}{m}", name=f"{nm}_sb{m}") for m in range(2)]
                for m in range(2):
                    nc.scalar.dma_start(out=b_sb[nm][m][:, :], in_=srcb[m * 128:(m + 1) * 128, :])
            for t in range(2):
                nc.scalar.dma_start(out=wo_sb[t][:, :], in_=wo[t * 128:(t + 1) * 128, :])
            make_identity(nc, ident[:, :])

            # ---- x staging tiles (8 chunk DMAs split across both rings;
            # a single rearranged whole-tensor DMA measures ~2x slower) ----
            def load_x(x_ap, nm, dmae=None, cols=None):
                tiles = []
                for k in range(8):
                    w = S if cols is None else 1024
                    xt = io.tile([128, w], bf16, tag=f"{nm}", name=f"{nm}{k}", bufs=8)
                    eng = nc.sync if k % 2 == 0 else nc.scalar
                    src = x_ap[k * 128:(k + 1) * 128, :] if cols is None else \
                        x_ap[k * 128:(k + 1) * 128, cols * 1024:(cols + 1) * 1024]
                    eng.dma_start(out=xt[:, :], in_=src)
                    tiles.append(xt)
                return tiles

            def load_w(w_ap, nm):
                w_sb = wp.tile([128, 8 * DLOC], bf16, tag="w", name=f"w_{nm}")
                nc.scalar.dma_start(
                    out=w_sb[:, :].rearrange("p (k m) -> p k m", k=8),
                    in_=w_ap.rearrange("(k p) m -> p k m", p=128),
                )
                return w_sb

            wk_sb = load_w(wk, "k")
            wq_sb = load_w(wq, "q")
            wv_sb = load_w(wv, "v")
            xk_t = load_x(xk, "xk", nc.sync)
            xqA = load_x(xq, "xqA", nc.scalar, cols=0)
            xv_t = load_x(xv, "xv", nc.sync)
            xqB = load_x(xq, "xqB", nc.scalar, cols=1)

            # ---- projection pass emitters ----------------------------------
            # proj n-pair: x chunks stream k-inner so matmuls start as DMA
            # lands; returns filler-unit closures of ~4-8 matmuls each.
            def proj_units(w_sb, x_tiles, dst, m, npair, bias, xoff=0, nm=""):
                st = {}

                def mms(klo, khi):
                    def f():
                        if "pacc" not in st:
                            st["pacc"] = [
                                ps.tile([128, 512], f32, tag="pp", name=f"pj{nm}{m}{n}")
                                for n in npair
                            ]
                        for k in range(klo, khi):
                            for i, n in enumerate(npair):
                                nc.tensor.matmul(
                                    st["pacc"][i][:, :],
                                    lhsT=w_sb[:, k * DLOC + m * 128: k * DLOC + (m + 1) * 128],
                                    rhs=x_tiles[k][:, (n - xoff) * 512:(n - xoff + 1) * 512],
                                    start=(k == 0), stop=(k == 7),
                                )
                    return f

                def evac():
                    for i, n in enumerate(npair):
                        nc.vector.tensor_scalar(
                            out=dst[m][:, n * 512:(n + 1) * 512],
                            in0=st["pacc"][i][:, :],
                            scalar1=1.0,
                            scalar2=bias[m][:, 0:1],
                            op0=MUL, op1=ADD,
                        )
                return [mms(0, 4), mms(4, 8), evac]

            # v projection, orientation A (features on partitions) + PE
            # transpose into the pair-packed [sk, feat] layout.  Returns two
            # unit-lists (one per n-pair), each with its transposes inline
            # right after the evac so vp quarters become valid (in trace
            # order) as early as possible.
            vt_bf = [None, None]

            def v_units(m):
                vt_bf[m] = vtp.tile([128, S], bf16, tag="vt", name=f"vt{m}")

                def transp(jc):
                    def f():
                        tr = ps.tile([128, 128], bf16, tag="pp", name=f"tr{m}{jc}")
                        nc.tensor.transpose(
                            tr[:, :], vt_bf[m][:, jc * 128:(jc + 1) * 128], ident[:, :]
                        )
                        nc.vector.tensor_copy(
                            out=vp[jc // 4][:, (jc % 4) * 256 + m * 128:
                                            (jc % 4) * 256 + m * 128 + 128],
                            in_=tr[:, :],
                        )
                    return f

                halves = []
                for hi, npair in enumerate(((0, 1), (2, 3))):
                    us = proj_units(wv_sb, xv_t, vt_bf, m, npair, b_sb["bv"], nm="v")
                    us += [transp(jc) for jc in range(hi * 8, hi * 8 + 8)]
                    halves.append(us)
                return halves

            def o_units(mbs):
                units = []

                def one(mb, n2):
                    def f():
                        sqh, col = mb // 8, (mb % 8) * 128
                        po = ps.tile([128, 512], f32, tag="pp", name=f"po{mb}{n2}")
                        for t in range(2):
                            nc.tensor.matmul(
                                po[:, :],
                                lhsT=ctxt[t][sqh][:, col:col + 128],
                                rhs=wo_sb[t][:, n2 * 512:(n2 + 1) * 512],
                                start=(t == 0), stop=(t == 1),
                            )
                        ev = oev.tile([128, 512], bf16, tag="oev", name=f"ev{mb}{n2}")
                        if mb < 8:
                            nc.vector.tensor_copy(out=ev[:, :], in_=po[:, :])
                            dmae = nc.sync
                        else:
                            nc.scalar.copy(out=ev[:, :], in_=po[:, :])
                            dmae = nc.sync if n2 == 0 else nc.scalar
                        dmae.dma_start(
                            out=out1[mb * 128:(mb + 1) * 128, n2 * 512:(n2 + 1) * 512],
                            in_=ev[:, :],
                        )
                    return f
                for mb in mbs:
                    for n2 in range(2):
                        units.append(one(mb, n2))
                return units

            # ---- attention pass (software-pipelined) -----------------------
            def emit_scores(t, sqh, j):
                sA = ps.tile([128, 1024], f32, tag="sc", name="sA")
                sB = ps.tile([128, 1024], f32, tag="sc", name="sB")
                # pair-adjacent: the two K=64 matmuls run concurrently on
                # row-groups 0-1 / 2-3 when both PSUM slots are free.
                for c in range(2):
                    for s_, lo in ((sA, 0), (sB, 64)):
                        nc.tensor.matmul(
                            s_[:, c * 512:(c + 1) * 512],
                            lhsT=kt[t][lo:lo + 64, j * 128:(j + 1) * 128],
                            rhs=qt[t][lo:lo + 64, sqh * 1024 + c * 512: sqh * 1024 + (c + 1) * 512],
                            start=True, stop=True,
                        )
                return sA, sB

            def emit_mask(sqh, j):
                mt = mpool.tile([128, 1024], bf16, tag="mt", name="mt")
                nc.sync.dma_start(
                    out=mt[:, :],
                    in_=mk[j * 128:(j + 1) * 128, sqh * 1024:(sqh + 1) * 1024],
                )
                return mt

            # One fused 64-step pipeline over (t, sqh, j): mask + scores are
            # emitted one step ahead ACROSS pass boundaries, so ACT never
            # waits at a boundary; ctx runs 2 steps deferred (both attn
            # tiles ready -> the M=64 pair runs concurrently); per-pass
            # filler units pace ~evenly through their 16 steps.
            def attn_all(seg_fillers):
                steps = [(t, sqh, j) for t in range(2) for sqh in range(2)
                         for j in range(16)]
                accs = {}
                ctx_q = []

                def emit_ctx(t, sqh, j, atnA, atnB):
                    acc = accs[(t, sqh)]
                    vbase = (j % 4) * 256 + t * 128
                    for c in range(2):
                        for hp, atn in ((0, atnA), (1, atnB)):
                            nc.tensor.matmul(
                                acc[hp * 64:(hp + 1) * 64, c * 512:(c + 1) * 512],
                                lhsT=vp[j // 4][:, vbase + hp * 64: vbase + hp * 64 + 64],
                                rhs=atn[:, c * 512:(c + 1) * 512],
                                start=(j == 0), stop=(j == 15),
                            )
                    if j == 15:
                        nc.vector.tensor_copy(out=ctxt[t][sqh][:, :], in_=acc[:, :])

                t0, sqh0, _ = steps[0]
                accs[(t0, sqh0)] = ps.tile([128, 1024], f32, tag="acc",
                                           name=f"acc{t0}{sqh0}", bufs=1)
                mt = emit_mask(sqh0, 0)
                sAB = emit_scores(t0, sqh0, 0)
                for i, (t, sqh, j) in enumerate(steps):
                    fillers = seg_fillers[2 * t + sqh]
                    sA, sB = sAB
                    attA = attp.tile([128, 1024], bf16, tag="att", name="attA", bufs=5)
                    attB = attp.tile([128, 1024], bf16, tag="att", name="attB", bufs=5)
                    nc.scalar.activation(attA[:, :], sA[:, :], SILU)
                    nc.scalar.activation(attB[:, :], sB[:, :], SILU)
                    atnA = attp.tile([128, 1024], bf16, tag="attn", name="atnA", bufs=11)
                    atnB = attp.tile([128, 1024], bf16, tag="attn", name="atnB", bufs=11)
                    nc.vector.tensor_mul(out=atnA[:, :], in0=attA[:, :], in1=mt[:, :])
                    nc.vector.tensor_mul(out=atnB[:, :], in0=attB[:, :], in1=mt[:, :])
                    if i + 1 < len(steps):
                        nt, nsqh, nj = steps[i + 1]
                        if nj == 0:
                            accs[(nt, nsqh)] = ps.tile([128, 1024], f32, tag="acc",
                                                       name=f"acc{nt}{nsqh}", bufs=1)
                        mt = emit_mask(nsqh, nj)
                        sAB = emit_scores(nt, nsqh, nj)
                    # filler units: pace the segment's list over its steps
                    rem_iters = 16 - j
                    n_emit = (len(fillers) + rem_iters - 1) // rem_iters if fillers else 0
                    for _ in range(min(n_emit, len(fillers))):
                        fillers.pop(0)()
                    ctx_q.append((t, sqh, j, atnA, atnB))
                    if len(ctx_q) > 2:
                        emit_ctx(*ctx_q.pop(0))
                while ctx_q:
                    emit_ctx(*ctx_q.pop(0))

            # ---- emission schedule ----------------------------------------
            # HAM warm-up: dep-free matmuls on garbage SBUF fill the
            # preamble + xk-DMA window with sustained PE activity so the
            # projections run at 2.4 GHz, not 1.2.  Output is discarded
            # (scores later overwrite the slot with start=True).
            wrm = ps.tile([128, 1024], f32, tag="sc", name="warm")
            for _ in range(24):
                nc.tensor.matmul(wrm[:, 0:512], lhsT=ident[:, :],
                                 rhs=kt[0][0:128, 0:512], start=True, stop=True)

            # head: k m0 + q m0 (cols 0-1023) gate the first scores
            for u in proj_units(wk_sb, xk_t, kt, 0, (0, 1), b_sb["bk"], nm="k"):
                u()
            for u in proj_units(wk_sb, xk_t, kt, 0, (2, 3), b_sb["bk"], nm="k"):
                u()
            for u in proj_units(wq_sb, xqA, qt, 0, (0, 1), b_sb["bq"], nm="q"):
                u()

            v0a, v0b = v_units(0)
            v1a, v1b = v_units(1)
            # head-start v m0's first mms before the attention stream
            v0a[0]()
            v0a[1]()
            # filler load balanced across segments.  Ordering rule: units
            # whose output is read by a NEXT-segment boundary score (k/q
            # evacs) come early; transposes only consumed by late deferred
            # ctx go last.  (v-pair-1's second half is first consumed by
            # ctx(1,0,8) around step 42, so it can pace through segment 2.)
            q023 = proj_units(wq_sb, xqB, qt, 0, (2, 3), b_sb["bq"], xoff=2, nm="q")
            q123 = proj_units(wq_sb, xqB, qt, 1, (2, 3), b_sb["bq"], xoff=2, nm="q")
            f00 = v0a[2:3] + v0a[3:7] + q023 + v0a[7:] + v0b
            f01 = v1a[:7] \
                + proj_units(wk_sb, xk_t, kt, 1, (0, 1), b_sb["bk"], nm="k") \
                + proj_units(wq_sb, xqA, qt, 1, (0, 1), b_sb["bq"], nm="q") \
                + v1a[7:]
            f10 = proj_units(wk_sb, xk_t, kt, 1, (2, 3), b_sb["bk"], nm="k") \
                + v1b[:7] + q123 + v1b[7:]
            f11 = [lambda: None] * 4 + o_units(range(0, 8))
            attn_all([f00, f01, f10, f11])
            for u in o_units(range(8, 16)):
                u()

    nc.compile()
    return nc


def get_program():
    if "nc" not in _COMPILED:
        _COMPILED["nc"] = build_program()
    return _COMPILED["nc"]


def make_in_maps(query, key, value, mask, Wq, bq, Wk, bk, Wv, bv, Wo):
    """Host-side sharding/layout prep: one input map per core."""
    query = np.asarray(query, dtype=F32)
    key = np.asarray(key, dtype=F32)
    value = np.asarray(value, dtype=F32)
    mask = np.asarray(mask)
    in_maps = []
    maskT = [np.ascontiguousarray(mask[b, 0].T).astype(BF16) for b in range(B)]
    xqT = [np.ascontiguousarray(query[b].T).astype(BF16) for b in range(B)]
    xkT = [np.ascontiguousarray(key[b].T).astype(BF16) for b in range(B)]
    xvT = [np.ascontiguousarray(value[b].T).astype(BF16) for b in range(B)]
    for c in range(N_CORES):
        b = c // GROUPS
        g = c % GROUPS
        rs = slice(g * DLOC, (g + 1) * DLOC)
        in_maps.append({
            "xq": xqT[b],
            "xk": xkT[b],
            "xv": xvT[b],
            "mk": maskT[b],
            "wq": np.ascontiguousarray((SCALE * np.asarray(Wq, F32))[rs, :].T).astype(BF16),
            "wk": np.ascontiguousarray(np.asarray(Wk, F32)[rs, :].T).astype(BF16),
            "wv": np.ascontiguousarray(np.asarray(Wv, F32)[rs, :].T).astype(BF16),
            "wo": np.ascontiguousarray(np.asarray(Wo, F32)[:, rs].T).astype(BF16),
            "bq": (SCALE * np.asarray(bq, F32)[rs]).reshape(DLOC, 1),
            "bk": np.asarray(bk, F32)[rs].reshape(DLOC, 1),
            "bv": np.asarray(bv, F32)[rs].reshape(DLOC, 1),
        })
    return in_maps


def run_on_device(in_maps, trace=False, tmpdir=None):
    from concourse.bass_utils import run_bass_kernel_spmd
    nc = get_program()
    kwargs = {}
    if trace:
        kwargs.update(trace=True, tmpdir=tmpdir)
    return run_bass_kernel_spmd(nc, in_maps, list(range(N_CORES)), **kwargs)


def assemble_output(results):
    out = np.zeros((B, S, HID), dtype=F32)
    for c in range(N_CORES):
        out[c // GROUPS] += results[c]["out1"].astype(F32)
    return out


def kernel(query, key, value, mask, Wq, bq, Wk, bk, Wv, bv, Wo):
    in_maps = make_in_maps(query, key, value, mask, Wq, bq, Wk, bk, Wv, bv, Wo)
    res = run_on_device(in_maps)
    return assemble_output(res.results)


# revision 49
# speedup vs baseline: 1.0467x; 1.0156x over previous
"""Trainium2 Bass kernel for CrossAttention (silu-scored, masked) sharded over
8 NeuronCores.

Problem (full shapes):
    query/key/value: [2, 2048, 1024] f32, mask: [2, 1, 2048, 2048] int32
    out = silu(mask((q @ k.T) * scale)) @ v heads-merged @ Wo.T

Sharding: core c handles batch b = c // 4 and heads 4*(c%4) .. 4*(c%4)+3
(data parallel on B, tensor parallel on heads).  Each core computes a
row-parallel partial of the O-projection; the host sums the 4 partials per
batch.  No cross-device communication.

Per-core program (all matmul operands bf16, f32 PSUM accumulate).  The 4
local heads form 2 pairs (even head on partitions 0-63, odd on 64-127 of
the shared qt/kt tiles).  Score matmuls are two K=64 row-group matmuls
(tile_position (0,0)/(64,0)) and context matmuls two M=64 col-group
matmuls ((0,0)/(0,64)), emitted pair-adjacent so they run CONCURRENTLY on
disjoint halves of the PE array — half the PE wall-clock of the padded
full-array formulation.

ACT (silu) is the roofline engine: 128 ACTIVATEs x ~1.1us ≈ 145us busy
(FD=1024 is forced by PSUM: score tiles must double-buffer next to the
ctx accumulator and a projection/O-proj scratch slot in 8 banks).  All
four (pair, sq-half) attention passes are fused into ONE 64-step
software pipeline so nothing gates ACT:
  - mask + scores for step i+1 are emitted before ctx of step i, across
    pass boundaries too (no boundary stall)
  - ctx runs 2 steps deferred (both attn tiles ready -> the two M=64
    col-group matmuls run concurrently)
  - projection work that isn't needed up front (v, k/q second halves,
    q col-half 2, O-proj head) is chopped into ~1-2us "filler" units
    paced into the pipeline after their producers / before consumers
  - masks/outputs ride the sync DMA ring; the scalar ring is quiet after
    the early x loads so the ACT sequencer only runs ACTIVATEs
  - dep-free warm-up matmuls bridge the DMA head so the projections hit
    the PE at 2.4 GHz (HAM un-throttled) instead of 1.2.
"""

import numpy as np
import ml_dtypes

B = 2
S = 2048
HID = 1024
HEADS = 16
DH = 64
N_CORES = 8
GROUPS = 4
NH_LOC = HEADS // GROUPS   # 4 heads per core
DLOC = NH_LOC * DH         # 256 local features
SCALE = DH ** -0.5

F32 = np.float32
BF16 = ml_dtypes.bfloat16

_COMPILED = {}


def build_program():
    import concourse.bass as bass
    import concourse.tile as tile
    from concourse import bacc, mybir
    from concourse.masks import make_identity

    f32 = mybir.dt.float32
    bf16 = mybir.dt.bfloat16

    nc = bacc.Bacc("TRN2", target_bir_lowering=False, debug=False,
                   enable_asserts=False, num_devices=N_CORES)

    xq = nc.dram_tensor("xq", [HID, S], bf16, kind="ExternalInput").ap()
    xk = nc.dram_tensor("xk", [HID, S], bf16, kind="ExternalInput").ap()
    xv = nc.dram_tensor("xv", [HID, S], bf16, kind="ExternalInput").ap()
    mk = nc.dram_tensor("mk", [S, S], bf16, kind="ExternalInput").ap()
    wq = nc.dram_tensor("wq", [HID, DLOC], bf16, kind="ExternalInput").ap()
    wk = nc.dram_tensor("wk", [HID, DLOC], bf16, kind="ExternalInput").ap()
    wv = nc.dram_tensor("wv", [HID, DLOC], bf16, kind="ExternalInput").ap()
    wo = nc.dram_tensor("wo", [DLOC, HID], bf16, kind="ExternalInput").ap()
    bq = nc.dram_tensor("bq", [DLOC, 1], f32, kind="ExternalInput").ap()
    bk = nc.dram_tensor("bk", [DLOC, 1], f32, kind="ExternalInput").ap()
    bv = nc.dram_tensor("bv", [DLOC, 1], f32, kind="ExternalInput").ap()
    out1 = nc.dram_tensor("out1", [S, HID], bf16, kind="ExternalOutput").ap()

    SILU = mybir.ActivationFunctionType.Silu
    MUL = mybir.AluOpType.mult
    ADD = mybir.AluOpType.add

    with tile.TileContext(nc) as tc:
        with (
            tc.tile_pool(name="res", bufs=1) as res,
            tc.tile_pool(name="io", bufs=8) as io,
            tc.tile_pool(name="wp", bufs=3) as wp,
            # PSUM: sc 2x[128,1024] (4 banks) + acc 1x[128,1024] (2) +
            # pp 2x[128,512] (2) = 8 banks exactly.
            tc.tile_pool(name="ps", bufs=2, space="PSUM") as ps,
            tc.tile_pool(name="attp", bufs=4) as attp,
            tc.tile_pool(name="mpool", bufs=4) as mpool,
            tc.tile_pool(name="vt", bufs=1) as vtp,
            tc.tile_pool(name="oev", bufs=4) as oev,
        ):
            # ---- resident SBUF tensors ----
            qt = [res.tile([128, S], bf16, tag=f"qt{t}", name=f"qt{t}") for t in range(2)]
            kt = [res.tile([128, S], bf16, tag=f"kt{t}", name=f"kt{t}") for t in range(2)]
            # vp quarter qi covers sk-tiles 4qi..4qi+3; col layout within:
            # (j%4)*256 + t*128 + hp*64 + d
            vp = [res.tile([128, 1024], bf16, tag=f"vp{qi}", name=f"vp{qi}") for qi in range(4)]
            # ctxt[t][sqh]: rows = pair-t features, cols = sq half
            ctxt = [[res.tile([128, 1024], bf16, tag=f"cx{t}{sqh}", name=f"cx{t}{sqh}")
                     for sqh in range(2)] for t in range(2)]
            wo_sb = [res.tile([128, HID], bf16, tag=f"wo{t}", name=f"wo_sb{t}") for t in range(2)]
            ident = res.tile([128, 128], bf16, tag="ident", name="ident")
            b_sb = {}
            for nm, srcb in (("bq", bq), ("bk", bk), ("bv", bv)):
                b_sb[nm] = [res.tile([128, 1], f32, tag=f"{nm}{m}", name=f"{nm}_sb{m}") for m in range(2)]
                for m in range(2):
                    nc.scalar.dma_start(out=b_sb[nm][m][:, :], in_=srcb[m * 128:(m + 1) * 128, :])
            for t in range(2):
                nc.scalar.dma_start(out=wo_sb[t][:, :], in_=wo[t * 128:(t + 1) * 128, :])
            make_identity(nc, ident[:, :])

            # ---- x staging tiles (8 chunk DMAs split across both rings;
            # a single rearranged whole-tensor DMA measures ~2x slower) ----
            def load_x(x_ap, nm, dmae=None, cols=None):
                tiles = []
                for k in range(8):
                    w = S if cols is None else 1024
                    xt = io.tile([128, w], bf16, tag=f"{nm}", name=f"{nm}{k}", bufs=8)
                    eng = nc.sync if k % 2 == 0 else nc.scalar
                    src = x_ap[k * 128:(k + 1) * 128, :] if cols is None else \
                        x_ap[k * 128:(k + 1) * 128, cols * 1024:(cols + 1) * 1024]
                    eng.dma_start(out=xt[:, :], in_=src)
                    tiles.append(xt)
                return tiles

            def load_w(w_ap, nm):
                w_sb = wp.tile([128, 8 * DLOC], bf16, tag="w", name=f"w_{nm}")
                nc.scalar.dma_start(
                    out=w_sb[:, :].rearrange("p (k m) -> p k m", k=8),
                    in_=w_ap.rearrange("(k p) m -> p k m", p=128),
                )
                return w_sb

            wk_sb = load_w(wk, "k")
            wq_sb = load_w(wq, "q")
            wv_sb = load_w(wv, "v")
            xk_t = load_x(xk, "xk", nc.sync)
            xqA = load_x(xq, "xqA", nc.scalar, cols=0)
            xv_t = load_x(xv, "xv", nc.sync)
            xqB = load_x(xq, "xqB", nc.scalar, cols=1)

            # ---- projection pass emitters ----------------------------------
            # proj n-pair: x chunks stream k-inner so matmuls start as DMA
            # lands; returns filler-unit closures of ~4-8 matmuls each.
            def proj_units(w_sb, x_tiles, dst, m, npair, bias, xoff=0, nm=""):
                st = {}

                def mms(klo, khi):
                    def f():
                        if "pacc" not in st:
                            st["pacc"] = [
                                ps.tile([128, 512], f32, tag="pp", name=f"pj{nm}{m}{n}")
                                for n in npair
                            ]
                        for k in range(klo, khi):
                            for i, n in enumerate(npair):
                                nc.tensor.matmul(
                                    st["pacc"][i][:, :],
                                    lhsT=w_sb[:, k * DLOC + m * 128: k * DLOC + (m + 1) * 128],
                                    rhs=x_tiles[k][:, (n - xoff) * 512:(n - xoff + 1) * 512],
                                    start=(k == 0), stop=(k == 7),
                                )
                    return f

                def evac():
                    for i, n in enumerate(npair):
                        nc.vector.tensor_scalar(
                            out=dst[m][:, n * 512:(n + 1) * 512],
                            in0=st["pacc"][i][:, :],
                            scalar1=1.0,
                            scalar2=bias[m][:, 0:1],
                            op0=MUL, op1=ADD,
                        )
                return [mms(0, 4), mms(4, 8), evac]

            # v projection, orientation A (features on partitions) + PE
            # transpose into the pair-packed [sk, feat] layout.  Returns two
            # unit-lists (one per n-pair), each with its transposes inline
            # right after the evac so vp quarters become valid (in trace
            # order) as early as possible.
            vt_bf = [None, None]

            def v_units(m):
                vt_bf[m] = vtp.tile([128, S], bf16, tag="vt", name=f"vt{m}")

                def transp(jc):
                    def f():
                        tr = ps.tile([128, 128], bf16, tag="pp", name=f"tr{m}{jc}")
                        nc.tensor.transpose(
                            tr[:, :], vt_bf[m][:, jc * 128:(jc + 1) * 128], ident[:, :]
                        )
                        nc.vector.tensor_copy(
                            out=vp[jc // 4][:, (jc % 4) * 256 + m * 128:
                                            (jc % 4) * 256 + m * 128 + 128],
                            in_=tr[:, :],
                        )
                    return f

                halves = []
                for hi, npair in enumerate(((0, 1), (2, 3))):
                    us = proj_units(wv_sb, xv_t, vt_bf, m, npair, b_sb["bv"], nm="v")
                    us += [transp(jc) for jc in range(hi * 8, hi * 8 + 8)]
                    halves.append(us)
                return halves

            def o_units(mbs):
                units = []

                def one(mb, n2):
                    def f():
                        sqh, col = mb // 8, (mb % 8) * 128
                        po = ps.tile([128, 512], f32, tag="pp", name=f"po{mb}{n2}")
                        for t in range(2):
                            nc.tensor.matmul(
                                po[:, :],
                                lhsT=ctxt[t][sqh][:, col:col + 128],
                                rhs=wo_sb[t][:, n2 * 512:(n2 + 1) * 512],
                                start=(t == 0), stop=(t == 1),
                            )
                        ev = oev.tile([128, 512], bf16, tag="oev", name=f"ev{mb}{n2}")
                        if mb < 8:
                            nc.vector.tensor_copy(out=ev[:, :], in_=po[:, :])
                            dmae = nc.sync
                        else:
                            nc.scalar.copy(out=ev[:, :], in_=po[:, :])
                            dmae = nc.sync if n2 == 0 else nc.scalar
                        dmae.dma_start(
                            out=out1[mb * 128:(mb + 1) * 128, n2 * 512:(n2 + 1) * 512],
                            in_=ev[:, :],
                        )
                    return f
                for mb in mbs:
                    for n2 in range(2):
                        units.append(one(mb, n2))
                return units

            # ---- attention pass (software-pipelined) -----------------------
            def emit_scores(t, sqh, j):
                sA = ps.tile([128, 1024], f32, tag="sc", name="sA")
                sB = ps.tile([128, 1024], f32, tag="sc", name="sB")
                # pair-adjacent: the two K=64 matmuls run concurrently on
                # row-groups 0-1 / 2-3 when both PSUM slots are free.
                for c in range(2):
                    for s_, lo in ((sA, 0), (sB, 64)):
                        nc.tensor.matmul(
                            s_[:, c * 512:(c + 1) * 512],
                            lhsT=kt[t][lo:lo + 64, j * 128:(j + 1) * 128],
                            rhs=qt[t][lo:lo + 64, sqh * 1024 + c * 512: sqh * 1024 + (c + 1) * 512],
                            start=True, stop=True,
                        )
                return sA, sB

            def emit_mask(sqh, j):
                mt = mpool.tile([128, 1024], bf16, tag="mt", name="mt")
                nc.sync.dma_start(
                    out=mt[:, :],
                    in_=mk[j * 128:(j + 1) * 128, sqh * 1024:(sqh + 1) * 1024],
                )
                return mt

            # One fused 64-step pipeline over (t, sqh, j): mask + scores are
            # emitted one step ahead ACROSS pass boundaries, so ACT never
            # waits at a boundary; ctx runs 2 steps deferred (both attn
            # tiles ready -> the M=64 pair runs concurrently); per-pass
            # filler units pace ~evenly through their 16 steps.
            def attn_all(seg_fillers):
                steps = [(t, sqh, j) for t in range(2) for sqh in range(2)
                         for j in range(16)]
                accs = {}
                ctx_q = []

                def emit_ctx(t, sqh, j, atnA, atnB):
                    acc = accs[(t, sqh)]
                    vbase = (j % 4) * 256 + t * 128
                    for c in range(2):
                        for hp, atn in ((0, atnA), (1, atnB)):
                            nc.tensor.matmul(
                                acc[hp * 64:(hp + 1) * 64, c * 512:(c + 1) * 512],
                                lhsT=vp[j // 4][:, vbase + hp * 64: vbase + hp * 64 + 64],
                                rhs=atn[:, c * 512:(c + 1) * 512],
                                start=(j == 0), stop=(j == 15),
                            )
                    if j == 15:
                        nc.vector.tensor_copy(out=ctxt[t][sqh][:, :], in_=acc[:, :])

                t0, sqh0, _ = steps[0]
                accs[(t0, sqh0)] = ps.tile([128, 1024], f32, tag="acc",
                                           name=f"acc{t0}{sqh0}", bufs=1)
                mt = emit_mask(sqh0, 0)
                sAB = emit_scores(t0, sqh0, 0)
                for i, (t, sqh, j) in enumerate(steps):
                    fillers = seg_fillers[2 * t + sqh]
                    sA, sB = sAB
                    attA = attp.tile([128, 1024], bf16, tag="att", name="attA", bufs=5)
                    attB = attp.tile([128, 1024], bf16, tag="att", name="attB", bufs=5)
                    nc.scalar.activation(attA[:, :], sA[:, :], SILU)
                    nc.scalar.activation(attB[:, :], sB[:, :], SILU)
                    atnA = attp.tile([128, 1024], bf16, tag="attn", name="atnA", bufs=11)
                    atnB = attp.tile([128, 1024], bf16, tag="attn", name="atnB", bufs=11)
                    nc.vector.tensor_mul(out=atnA[:, :], in0=attA[:, :], in1=mt[:, :])
                    nc.vector.tensor_mul(out=atnB[:, :], in0=attB[:, :], in1=mt[:, :])
                    if i + 1 < len(steps):
                        nt, nsqh, nj = steps[i + 1]
                        if nj == 0:
                            accs[(nt, nsqh)] = ps.tile([128, 1024], f32, tag="acc",
                                                       name=f"acc{nt}{nsqh}", bufs=1)
                        mt = emit_mask(nsqh, nj)
                        sAB = emit_scores(nt, nsqh, nj)
                    # filler units: pace the segment's list over its steps
                    rem_iters = 16 - j
                    n_emit = (len(fillers) + rem_iters - 1) // rem_iters if fillers else 0
                    for _ in range(min(n_emit, len(fillers))):
                        fillers.pop(0)()
                    ctx_q.append((t, sqh, j, atnA, atnB))
                    if len(ctx_q) > 2:
                        emit_ctx(*ctx_q.pop(0))
                while ctx_q:
                    emit_ctx(*ctx_q.pop(0))

            # ---- emission schedule ----------------------------------------
            # HAM warm-up: dep-free matmuls on garbage SBUF fill the
            # preamble + xk-DMA window with sustained PE activity so the
            # projections run at 2.4 GHz, not 1.2.  Output is discarded
            # (scores later overwrite the slot with start=True).
            wrm = ps.tile([128, 1024], f32, tag="sc", name="warm")
            for _ in range(30):
                nc.tensor.matmul(wrm[:, 0:512], lhsT=ident[:, :],
                                 rhs=kt[0][0:128, 0:512], start=True, stop=True)

            # head: k m0 + q m0 (cols 0-1023) gate the first scores
            for u in proj_units(wk_sb, xk_t, kt, 0, (0, 1), b_sb["bk"], nm="k"):
                u()
            for u in proj_units(wk_sb, xk_t, kt, 0, (2, 3), b_sb["bk"], nm="k"):
                u()
            for u in proj_units(wq_sb, xqA, qt, 0, (0, 1), b_sb["bq"], nm="q"):
                u()

            v0a, v0b = v_units(0)
            v1a, v1b = v_units(1)
            # head-start v m0's first mms before the attention stream
            v0a[0]()
            v0a[1]()
            # filler load balanced across segments.  Ordering rule: units
            # whose output is read by a NEXT-segment boundary score (k/q
            # evacs) come early; transposes only consumed by late deferred
            # ctx go last.  (v-pair-1's second half is first consumed by
            # ctx(1,0,8) around step 42, so it can pace through segment 2.)
            q023 = proj_units(wq_sb, xqB, qt, 0, (2, 3), b_sb["bq"], xoff=2, nm="q")
            q123 = proj_units(wq_sb, xqB, qt, 1, (2, 3), b_sb["bq"], xoff=2, nm="q")
            f00 = v0a[2:3] + v0a[3:7] + q023 + v0a[7:] + v0b
            f01 = v1a[:7] \
                + proj_units(wk_sb, xk_t, kt, 1, (0, 1), b_sb["bk"], nm="k") \
                + proj_units(wq_sb, xqA, qt, 1, (0, 1), b_sb["bq"], nm="q") \
                + v1a[7:]
            f10 = proj_units(wk_sb, xk_t, kt, 1, (2, 3), b_sb["bk"], nm="k") \
                + v1b[:7] + q123 + v1b[7:]
            f11 = [lambda: None] * 4 + o_units(range(0, 8))
            attn_all([f00, f01, f10, f11])
            for u in o_units(range(8, 16)):
                u()

    nc.compile()
    return nc


def get_program():
    if "nc" not in _COMPILED:
        _COMPILED["nc"] = build_program()
    return _COMPILED["nc"]


def make_in_maps(query, key, value, mask, Wq, bq, Wk, bk, Wv, bv, Wo):
    """Host-side sharding/layout prep: one input map per core."""
    query = np.asarray(query, dtype=F32)
    key = np.asarray(key, dtype=F32)
    value = np.asarray(value, dtype=F32)
    mask = np.asarray(mask)
    in_maps = []
    maskT = [np.ascontiguousarray(mask[b, 0].T).astype(BF16) for b in range(B)]
    xqT = [np.ascontiguousarray(query[b].T).astype(BF16) for b in range(B)]
    xkT = [np.ascontiguousarray(key[b].T).astype(BF16) for b in range(B)]
    xvT = [np.ascontiguousarray(value[b].T).astype(BF16) for b in range(B)]
    for c in range(N_CORES):
        b = c // GROUPS
        g = c % GROUPS
        rs = slice(g * DLOC, (g + 1) * DLOC)
        in_maps.append({
            "xq": xqT[b],
            "xk": xkT[b],
            "xv": xvT[b],
            "mk": maskT[b],
            "wq": np.ascontiguousarray((SCALE * np.asarray(Wq, F32))[rs, :].T).astype(BF16),
            "wk": np.ascontiguousarray(np.asarray(Wk, F32)[rs, :].T).astype(BF16),
            "wv": np.ascontiguousarray(np.asarray(Wv, F32)[rs, :].T).astype(BF16),
            "wo": np.ascontiguousarray(np.asarray(Wo, F32)[:, rs].T).astype(BF16),
            "bq": (SCALE * np.asarray(bq, F32)[rs]).reshape(DLOC, 1),
            "bk": np.asarray(bk, F32)[rs].reshape(DLOC, 1),
            "bv": np.asarray(bv, F32)[rs].reshape(DLOC, 1),
        })
    return in_maps


def run_on_device(in_maps, trace=False, tmpdir=None):
    from concourse.bass_utils import run_bass_kernel_spmd
    nc = get_program()
    kwargs = {}
    if trace:
        kwargs.update(trace=True, tmpdir=tmpdir)
    return run_bass_kernel_spmd(nc, in_maps, list(range(N_CORES)), **kwargs)


def assemble_output(results):
    out = np.zeros((B, S, HID), dtype=F32)
    for c in range(N_CORES):
        out[c // GROUPS] += results[c]["out1"].astype(F32)
    return out


def kernel(query, key, value, mask, Wq, bq, Wk, bk, Wv, bv, Wo):
    in_maps = make_in_maps(query, key, value, mask, Wq, bq, Wk, bk, Wv, bv, Wo)
    res = run_on_device(in_maps)
    return assemble_output(res.results)
